# revision 19
# baseline (speedup 1.0000x reference)
"""CapsuleTransformConv on 8 Trainium2 NeuronCores.

Problem:  x [4,16,16,32,16] f32, matrix [288,16,512] f32.
          im2col (K=3, VALID) -> tile [4,14,14,288,16]
          votes  = einsum('bhwna,nac->bhwnc', tile, matrix)
          out    = votes.reshape(4,14,14,288,32,16)

Sharding: tensor-parallel over the filter*atom output axis (512 -> 64 per
core).  Every core reads the full x and its 64-wide weight slice; writes
its [784, 288, 64] output slice (~29 MB bf16, the dominant HBM traffic).

Per-core kernel structure (v3):
  - Host marshals inputs: x is pre-transposed to 4 per-octet fp16 tiles
    xt[oct][(dc,a)=128, (b,h,w)=1024] and the weights are pre-packed into
    9 block-diagonal fp16 tiles wp[kk][(gc,a)=128, oct*512+gc*64+f] (one
    K=128 matmul computes 8 independent [pos,16]@[16,64] capsule matmuls).
    This removes the on-device PE transposes, scattered weight-paint DMAs
    and f32r rounding copies entirely.
  - Instead of 9 per-tap im2col compactions, GPSIMD builds only 3
    kj-shifted tensors shift[kj][(dc,a), (oct,b,h16,j14)]; the three ki
    taps of a kj slice them as contiguous row windows (walrus needs a
    flat stationary slice).  3x less GPSIMD gather work, built one kj
    ahead, per-(b,oct) granularity so the first matmul starts ~3us in.
  - Main loop: 3 kj x 3 ki x (4 b x 2 i-windows); 4 matmuls (c-octets)
    into a double-buffered 4-bank PSUM tile, PSUM->SBUF drain split
    DVE[0:960] || ACT[960:2048] converting f32 -> bf16, then one
    contiguous ~0.2-0.3 MB DMA per window to the tap-major bf16 output.
  - All output DMAs issue on the Sync queue only: a dma_start on the ACT
    queue waits on the DVE half-drain and stalls the next ACT drain
    behind it (in-order queue), which was the v2 serializer.
  - fp16 matmul inputs (1 cyc/row), f32 PSUM accumulate, bf16 output
    write; host upcasts to f32.  rel err ~2e-3 vs the f32 reference.
"""

import numpy as np

B, H, W, C, A = 4, 16, 16, 32, 16
KS = 3
OH = OW = 14
NCAP = KS * KS * C          # 288 capsules
FTOT = 512                  # filter*atom
NCORES = 8
FPC = FTOT // NCORES        # 64 output features per core
POS = B * OH * OW           # 784 output positions

_NC_CACHE = {}


def _build_nc():
    import concourse.bass as bass  # noqa: F401
    import concourse.mybir as mybir
    import concourse.tile as tile
    from concourse import bacc

    f32 = mybir.dt.float32
    fp16 = mybir.dt.float16
    bf16 = mybir.dt.bfloat16

    nc = bacc.Bacc(None, target_bir_lowering=False)
    xt_d = nc.declare_dram_parameter("xt", [4, 128, B * H * W], fp16,
                                     isOutput=False)
    wp_d = nc.declare_dram_parameter("wp", [KS * KS, 128, 4 * 512], fp16,
                                     isOutput=False)
    # Tap-major output layout: out[kk, pos=784, 32*64].  Valid-only windows
    # (112/84 rows per batch) keep every DMA source partition-offset-0:
    # offset sources triple the HWDGE DIRECT2D descriptor-generation cost,
    # and garbage rows would add 14% to the saturated DMA write traffic.
    NS = B * H * OW             # 896 flat (b,h,j) rows per shift octet
    o_d = nc.declare_dram_parameter("out", [KS * KS, POS, 32 * FPC], bf16,
                                    isOutput=True)

    with tile.TileContext(nc) as tc:
        with (
            tc.tile_pool(name="xtp", bufs=1) as xtp,
            tc.tile_pool(name="wpp", bufs=1) as wpp,
            tc.tile_pool(name="shiftp", bufs=2) as shiftp,
            tc.tile_pool(name="stage", bufs=8) as stagep,
            tc.tile_pool(name="psum", bufs=4, space="PSUM") as psump,
        ):
            # ---- weight packs: first-consumed tap first, on the ACT ring
            # (the sync ring carries xt + all output DMAs) ----
            kk_order = [ki * 3 + kj for kj in range(3) for ki in range(3)]
            wps = [None] * (KS * KS)
            for kk in kk_order:
                wp_t = wpp.tile([128, 4 * 512], fp16, tag=f"wp{kk}",
                                name=f"wp{kk}")
                nc.scalar.dma_start(wp_t[:], wp_d[kk])
                wps[kk] = wp_t

            # ---- x: already transposed+fp16 on host; 4 per-octet tiles ----
            xts = [
                xtp.tile([128, B * H * W], fp16, tag=f"xt{o}", name=f"xt{o}")
                for o in range(4)
            ]
            for o in range(4):
                nc.sync.dma_start(xts[o][:], xt_d[o])
            xtvs = [
                t[:].rearrange("p (b h w) -> p b h w", b=B, h=H) for t in xts
            ]

            # ---- kj-shift builds: shift[(dc,a), (oct,b,h,j)] ----
            # j:14-of-16 compaction only; the three ki taps of this kj read
            # contiguous row windows.  Per-(b,oct) GPSIMD copies so early
            # matmuls start as soon as their slice lands.
            def build_shift(kj, first=False):
                sh = shiftp.tile([128, 4 * B * H * OW], fp16, tag="shift",
                                 name=f"shift{kj}")
                shv = sh[:].rearrange("p (o b h j) -> p o b h j", o=4, b=B,
                                      h=H)
                for b in range(B):
                    for o in range(4):
                        # At startup DVE/ACT are idle: let them build the
                        # first batch's chunks so matmuls start sooner.
                        if first and b == 0 and o < 2:
                            eng = nc.vector.tensor_copy if o == 0 \
                                else nc.scalar.copy
                        else:
                            eng = nc.gpsimd.tensor_copy
                        eng(shv[:, o, b], xtvs[o][:, b, :, kj:kj + OW])
                return sh

            # ---- main loop: kj outer so taps ki=0..2 reuse one shift ----
            # 7 full 128-partition windows slide over each octet's 896-wide
            # (b,h,j) space; h-out-of-range rows are computed and written
            # but stripped on host (drain cost is free-dim-bound and the
            # aligned full-block DMA is cheap to generate).
            # Each window drains on ONE engine (alternating DVE/ACT): the
            # two halves then carry no cross-engine stage-tile WAW dep,
            # which previously serialized the two drain engines.  Per-half
            # PSUM tiles (2 banks, bufs=4) let the first drain start after
            # matmul oct1, keeping PSUM turnaround off the critical path.
            sh_cur = build_shift(0, first=True)
            # Greedy drain-engine balance: ACT is faster per whole-window
            # drain (2x(172+1024)/1.2GHz = 2.0us) than DVE (2x(120+1024)/
            # 0.96GHz = 2.4us), so ACT takes ~4 of every 7 windows.
            acc = {"a": 0.0, "v": 0.0}
            for kj in range(3):
                sh_next = build_shift(kj + 1) if kj < 2 else None
                for ki in range(3):
                    kk = ki * 3 + kj
                    for b in range(B):
                        for i0, ni in ((0, 8), (8, 6)):
                            m = ni * OW  # 112 or 84 valid rows
                            psa = psump.tile([128, 1024], f32, tag="mm")
                            psb = psump.tile([128, 1024], f32, tag="mm")
                            for o in range(4):
                                ps = psa if o < 2 else psb
                                off = ((o * B + b) * H + ki + i0) * OW
                                nc.tensor.matmul(
                                    ps[0:m, (o % 2) * 512:(o % 2) * 512 + 512],
                                    sh_cur[:, off: off + m],
                                    wps[kk][:, o * 512:(o + 1) * 512],
                                    start=True,
                                    stop=True,
                                )
                            st = stagep.tile([128, 2048], bf16, tag="st")
                            # ACT-drained windows also issue their DMA on
                            # the ACT HWDGE ring (same queue -> no cross-
                            # engine wait, and it halves per-ring DIRECT2D
                            # descriptor-generation load).  Cost model:
                            # ACT 2x(172+1024)/1.2 + ~0.9 D2D; DVE
                            # 2x(120+1024)/0.96.
                            if acc["a"] + 2.9 <= acc["v"] + 2.4:
                                eng, dma_eng, key, cost = (
                                    nc.scalar.copy, nc.scalar, "a", 2.9)
                            else:
                                eng, dma_eng, key, cost = (
                                    nc.vector.tensor_copy, nc.sync, "v", 2.4)
                            acc[key] += cost
                            eng(st[0:m, 0:1024], psa[0:m, :])
                            eng(st[0:m, 1024:2048], psb[0:m, :])
                            q0 = b * (OH * OW) + i0 * OW
                            dma_eng.dma_start(
                                o_d[kk, q0: q0 + m, :], st[0:m, :]
                            )
                sh_cur = sh_next

    nc.compile()
    return nc


def _get_nc():
    if "v3" not in _NC_CACHE:
        _NC_CACHE["v3"] = _build_nc()
    return _NC_CACHE["v3"]


def make_in_maps(x, matrix):
    """Host-side input marshalling for all 8 cores."""
    x = np.ascontiguousarray(x, dtype=np.float32)
    matrix = np.ascontiguousarray(matrix, dtype=np.float32)
    # xt[oct, (dc,a), (b,h,w)] fp16
    xt = np.ascontiguousarray(
        x.transpose(3, 4, 0, 1, 2).reshape(4, 128, B * H * W)
    ).astype(np.float16)
    in_maps = []
    for c in range(NCORES):
        mr = matrix[:, :, c * FPC:(c + 1) * FPC]          # [288, 16, 64]
        mr = mr.reshape(KS * KS, 4, 8, A, FPC)            # [kk,oct,gc,a,f]
        wp = np.zeros((KS * KS, 128, 4 * 512), np.float16)
        wpv = wp.reshape(KS * KS, 8, A, 4, 8, FPC)        # [kk,gc,a,oct,gc2,f]
        for g in range(8):
            wpv[:, g, :, :, g, :] = mr[:, :, g].transpose(0, 2, 1, 3)
        in_maps.append({"xt": xt, "wp": wp})
    return in_maps


def kernel(x, matrix):
    from concourse.bass_utils import run_bass_kernel_spmd

    nc = _get_nc()
    in_maps = make_in_maps(x, matrix)
    r = run_bass_kernel_spmd(nc, in_maps, list(range(NCORES)))
    # parts[c]: [9, 784, 2048] tap-major -> [pos, kk, 32, core, 64] -> full
    arr = np.stack(
        [np.asarray(r.results[c]["out"]).astype(np.float32) for c in range(NCORES)]
    )
    arr = arr.reshape(NCORES, KS * KS, POS, 32, FPC)
    arr = arr.transpose(2, 1, 3, 0, 4)               # [pos, kk, 32, core, f]
    full = arr.reshape(POS, NCAP, FTOT)
    return np.ascontiguousarray(
        full.reshape(B, OH, OW, NCAP, 32, 16).astype(np.float32)
    )


# revision 22
# speedup vs baseline: 1.2536x; 1.2536x over previous
"""CapsuleTransformConv on 8 Trainium2 NeuronCores.

Problem:  x [4,16,16,32,16] f32, matrix [288,16,512] f32.
          im2col (K=3, VALID) -> tile [4,14,14,288,16]
          votes  = einsum('bhwna,nac->bhwnc', tile, matrix)
          out    = votes.reshape(4,14,14,288,32,16)

Sharding: tensor-parallel over the filter*atom output axis (512 -> 64 per
core).  Every core reads the full x and its 64-wide weight slice; writes
its [784, 288, 64] output slice (~29 MB bf16, the dominant HBM traffic).

Per-core kernel structure (v3):
  - Host marshals inputs: x is pre-transposed to 4 per-octet fp16 tiles
    xt[oct][(dc,a)=128, (b,h,w)=1024] and the weights are pre-packed into
    9 block-diagonal fp16 tiles wp[kk][(gc,a)=128, oct*512+gc*64+f] (one
    K=128 matmul computes 8 independent [pos,16]@[16,64] capsule matmuls).
    This removes the on-device PE transposes, scattered weight-paint DMAs
    and f32r rounding copies entirely.
  - Instead of 9 per-tap im2col compactions, GPSIMD builds only 3
    kj-shifted tensors shift[kj][(dc,a), (oct,b,h16,j14)]; the three ki
    taps of a kj slice them as contiguous row windows (walrus needs a
    flat stationary slice).  3x less GPSIMD gather work, built one kj
    ahead, per-(b,oct) granularity so the first matmul starts ~3us in.
  - Main loop: 3 kj x 3 ki x (4 b x 2 i-windows); 4 matmuls (c-octets)
    into a double-buffered 4-bank PSUM tile, PSUM->SBUF drain split
    DVE[0:960] || ACT[960:2048] converting f32 -> bf16, then one
    contiguous ~0.2-0.3 MB DMA per window to the tap-major bf16 output.
  - All output DMAs issue on the Sync queue only: a dma_start on the ACT
    queue waits on the DVE half-drain and stalls the next ACT drain
    behind it (in-order queue), which was the v2 serializer.
  - fp16 matmul inputs (1 cyc/row), f32 PSUM accumulate, bf16 output
    write; host upcasts to f32.  rel err ~2e-3 vs the f32 reference.
"""

import numpy as np

B, H, W, C, A = 4, 16, 16, 32, 16
KS = 3
OH = OW = 14
NCAP = KS * KS * C          # 288 capsules
FTOT = 512                  # filter*atom
NCORES = 8
FPC = FTOT // NCORES        # 64 output features per core
POS = B * OH * OW           # 784 output positions

_NC_CACHE = {}


def _build_nc():
    import concourse.bass as bass  # noqa: F401
    import concourse.mybir as mybir
    import concourse.tile as tile
    from concourse import bacc

    f32 = mybir.dt.float32
    fp16 = mybir.dt.float16
    bf16 = mybir.dt.bfloat16

    nc = bacc.Bacc(None, target_bir_lowering=False)
    xt_d = nc.declare_dram_parameter("xt", [4, 128, B * H * W], fp16,
                                     isOutput=False)
    wp_d = nc.declare_dram_parameter("wp", [KS * KS, 128, 4 * 512], fp16,
                                     isOutput=False)
    # Wrapped tap-major output layout: out[kk, (b,h,j)=896, 32*64].  All 128
    # rows of every window are written (including the ~14% h-out-of-range
    # garbage rows); the host strips them.  Full offset-0 [128,2048] blocks
    # keep the HWDGE DIRECT2D descriptor generation cheap -- valid-only
    # variants (112/84-row or offset-source DMAs) measured slower because
    # descriptor generation became the serializer.
    NS = B * H * OW             # 896 wrapped rows per tap
    o_d = nc.declare_dram_parameter("out", [KS * KS, NS, 32 * FPC], bf16,
                                    isOutput=True)

    with tile.TileContext(nc) as tc:
        with (
            tc.tile_pool(name="xtp", bufs=1) as xtp,
            tc.tile_pool(name="wpp", bufs=1) as wpp,
            tc.tile_pool(name="shiftp", bufs=2) as shiftp,
            tc.tile_pool(name="stage", bufs=8) as stagep,
            tc.tile_pool(name="psum", bufs=4, space="PSUM") as psump,
        ):
            # ---- weight packs: first-consumed tap first, on the ACT ring
            # (the sync ring carries xt + all output DMAs) ----
            kk_order = [ki * 3 + kj for kj in range(3) for ki in range(3)]
            wps = [None] * (KS * KS)
            for kk in kk_order:
                wp_t = wpp.tile([128, 4 * 512], fp16, tag=f"wp{kk}",
                                name=f"wp{kk}")
                nc.scalar.dma_start(wp_t[:], wp_d[kk])
                wps[kk] = wp_t

            # ---- x: already transposed+fp16 on host; 4 per-octet tiles ----
            xts = [
                xtp.tile([128, B * H * W], fp16, tag=f"xt{o}", name=f"xt{o}")
                for o in range(4)
            ]
            for o in range(4):
                nc.sync.dma_start(xts[o][:], xt_d[o])
            xtvs = [
                t[:].rearrange("p (b h w) -> p b h w", b=B, h=H) for t in xts
            ]

            # ---- kj-shift builds: shift[(dc,a), (oct,b,h,j)] ----
            # j:14-of-16 compaction only; the three ki taps of this kj read
            # contiguous row windows.  Per-(b,oct) GPSIMD copies so early
            # matmuls start as soon as their slice lands.
            def build_shift(kj, first=False):
                sh = shiftp.tile([128, 4 * B * H * OW], fp16, tag="shift",
                                 name=f"shift{kj}")
                shv = sh[:].rearrange("p (o b h j) -> p o b h j", o=4, b=B,
                                      h=H)
                for b in range(B):
                    for o in range(4):
                        # At startup DVE/ACT are idle: let them build the
                        # first batch's chunks so matmuls start sooner.
                        if first and b == 0 and o < 2:
                            eng = nc.vector.tensor_copy if o == 0 \
                                else nc.scalar.copy
                        else:
                            eng = nc.gpsimd.tensor_copy
                        eng(shv[:, o, b], xtvs[o][:, b, :, kj:kj + OW])
                return sh

            # ---- main loop: kj outer so taps ki=0..2 reuse one shift ----
            # 7 full 128-partition windows slide over each octet's 896-wide
            # (b,h,j) space; h-out-of-range rows are computed and written
            # but stripped on host (drain cost is free-dim-bound and the
            # aligned full-block DMA is cheap to generate).
            # Each window drains on ONE engine (alternating DVE/ACT): the
            # two halves then carry no cross-engine stage-tile WAW dep,
            # which previously serialized the two drain engines.  Per-half
            # PSUM tiles (2 banks, bufs=4) let the first drain start after
            # matmul oct1, keeping PSUM turnaround off the critical path.
            sh_cur = build_shift(0, first=True)
            # Greedy drain-engine balance: ACT is faster per whole-window
            # drain (2x(172+1024)/1.2GHz = 2.0us) than DVE (2x(120+1024)/
            # 0.96GHz = 2.4us), so ACT takes ~4 of every 7 windows.
            acc = {"a": 0.0, "v": 0.0}
            for kj in range(3):
                sh_next = build_shift(kj + 1) if kj < 2 else None
                for ki in range(3):
                    kk = ki * 3 + kj
                    for w in range(7):
                        psa = psump.tile([128, 1024], f32, tag="mm")
                        psb = psump.tile([128, 1024], f32, tag="mm")
                        for o in range(4):
                            ps = psa if o < 2 else psb
                            off = o * NS + 128 * w
                            nc.tensor.matmul(
                                ps[:, (o % 2) * 512:(o % 2) * 512 + 512],
                                sh_cur[:, off: off + 128],
                                wps[kk][:, o * 512:(o + 1) * 512],
                                start=True,
                                stop=True,
                            )
                        st = stagep.tile([128, 2048], bf16, tag="st")
                        if acc["a"] + 2.0 <= acc["v"] + 2.4:
                            eng, key, cost = nc.scalar.copy, "a", 2.0
                        else:
                            eng, key, cost = nc.vector.tensor_copy, "v", 2.4
                        acc[key] += cost
                        eng(st[:, 0:1024], psa[:])
                        eng(st[:, 1024:2048], psb[:])
                        nc.sync.dma_start(
                            o_d[kk, 128 * w: 128 * w + 128, :], st[:]
                        )
                sh_cur = sh_next

    nc.compile()
    return nc


def _get_nc():
    if "v3" not in _NC_CACHE:
        _NC_CACHE["v3"] = _build_nc()
    return _NC_CACHE["v3"]


def make_in_maps(x, matrix):
    """Host-side input marshalling for all 8 cores."""
    x = np.ascontiguousarray(x, dtype=np.float32)
    matrix = np.ascontiguousarray(matrix, dtype=np.float32)
    # xt[oct, (dc,a), (b,h,w)] fp16
    xt = np.ascontiguousarray(
        x.transpose(3, 4, 0, 1, 2).reshape(4, 128, B * H * W)
    ).astype(np.float16)
    in_maps = []
    for c in range(NCORES):
        mr = matrix[:, :, c * FPC:(c + 1) * FPC]          # [288, 16, 64]
        mr = mr.reshape(KS * KS, 4, 8, A, FPC)            # [kk,oct,gc,a,f]
        wp = np.zeros((KS * KS, 128, 4 * 512), np.float16)
        wpv = wp.reshape(KS * KS, 8, A, 4, 8, FPC)        # [kk,gc,a,oct,gc2,f]
        for g in range(8):
            wpv[:, g, :, :, g, :] = mr[:, :, g].transpose(0, 2, 1, 3)
        in_maps.append({"xt": xt, "wp": wp})
    return in_maps


def kernel(x, matrix):
    from concourse.bass_utils import run_bass_kernel_spmd

    nc = _get_nc()
    in_maps = make_in_maps(x, matrix)
    r = run_bass_kernel_spmd(nc, in_maps, list(range(NCORES)))
    # parts[c]: [9, 896, 2048] wrapped tap-major; strip the h-out-of-range
    # rows per tap (valid h for tap ki is [ki, ki+13]), then
    # [pos, kk, 32, core, 64] -> full
    arr = np.stack(
        [np.asarray(r.results[c]["out"]).astype(np.float32) for c in range(NCORES)]
    )
    arr = arr.reshape(NCORES, KS * KS, B, H * OW, 32, FPC)
    arr = np.stack(
        [arr[:, kk, :, OW * (kk // 3): OW * (kk // 3) + OH * OW]
         for kk in range(KS * KS)],
        axis=1,
    )                                                # [core, kk, b, 196, 32, f]
    arr = arr.reshape(NCORES, KS * KS, POS, 32, FPC)
    arr = arr.transpose(2, 1, 3, 0, 4)               # [pos, kk, 32, core, f]
    full = arr.reshape(POS, NCAP, FTOT)
    return np.ascontiguousarray(
        full.reshape(B, OH, OW, NCAP, 32, 16).astype(np.float32)
    )
